# revision 33
# baseline (speedup 1.0000x reference)
"""Trainium2 Bass kernel: causal attention with weight-normed QKV projections.

Problem (hardcoded): B=8, Cq=Ck=256, C=512, H=W=32 -> S=1024, N_HEAD=8, dh=64.
Sharding: pure data-parallel over batch (8 batches -> 8 cores), weights
replicated. No collectives.

v3 structure (trace-driven rework of the 98us baseline):
  - inputs bf16 + host-pretransposed p-major so every DMA moves large
    contiguous per-partition runs; vw split so Q/K weights land first.
  - weight prep: Q/K transposes run immediately on DMA arrival (scale is
    applied in the projection epilogue, so the norm chain is off the
    critical path); sum-of-squares on Pool (tensor_tensor_reduce), sqrt on
    ACT, recip on DVE, V-diag build on Pool.
  - V projection interleaved into the a2=0 attention loop (PE filler while
    ACT runs the first exps; VP ready before the a2=0 PV drain at a2=1).
  - engine split in main loop: PE matmuls / ACT exp only / DVE psum
    evacuation (projection epilogues, PV normalize, V bias) / Pool
    causal-mask multiplies (SBUF->SBUF bf16; Pool cannot touch PSUM).
  - PV normalize: one merged [128,2,64] tensor_tensor per (pair, qtile)
    with a stride-0 broadcast reciprocal operand.
  - output bf16 [S, C]; per-tile stores issued as soon as complete.
"""

import numpy as np

import concourse.bass as bass
import concourse.tile as tile
from concourse import bacc, mybir
from concourse.bass_utils import run_bass_kernel_spmd

F32 = mybir.dt.float32
BF16 = mybir.dt.bfloat16
AF = mybir.ActivationFunctionType
ALU = mybir.AluOpType

S = 1024          # sequence length (32*32)
CIN = 256         # input channels (Cq = Ck)
C = 512           # projection channels
NH = 8            # heads
DH = 64           # head dim
HW = 32           # spatial H = W
N_CORES = 8


def _build_module():
    nc = bacc.Bacc("TRN2", target_bir_lowering=False)

    vw_d = nc.dram_tensor("vw", [128, 3, 4, CIN], BF16, kind="ExternalInput").ap()
    qk_d = nc.dram_tensor("qk", [128, 4, S], BF16, kind="ExternalInput").ap()
    gb_d = nc.dram_tensor("gb", [128, 20], F32, kind="ExternalInput").ap()
    bv_d = nc.dram_tensor("bv", [C], BF16, kind="ExternalInput").ap()
    msk_d = nc.dram_tensor("msk", [128, 256], BF16, kind="ExternalInput").ap()
    o_d = nc.dram_tensor("o", [S, C], BF16, kind="ExternalOutput").ap()

    with tile.TileContext(nc) as tc:
        with (
            tc.tile_pool(name="const", bufs=1) as const,
            tc.tile_pool(name="persist", bufs=1) as persist,
            tc.tile_pool(name="wtmp", bufs=2) as wtmp,
            tc.tile_pool(name="smalls", bufs=4) as smalls,
        ):
            # ---- input DMAs: 3 rings, ordered by criticality
            msk_sb = const.tile([128, 256], BF16, name="msk_sb")
            nc.sync.dma_start(out=msk_sb, in_=msk_d)
            eye_bf = msk_sb[:, 0:128]
            triu = msk_sb[:, 128:256]
            # vw[p, t, c, i] = v_t[128c + p, i]; Q/K weights first
            vw_sb = persist.tile([128, 3, 4, CIN], BF16, name="vw_sb")
            nc.scalar.dma_start(out=vw_sb[:, 0:2, :, :], in_=vw_d[:, 0:2, :, :])
            nc.scalar.dma_start(out=vw_sb[:, 2:3, :, :], in_=vw_d[:, 2:3, :, :])
            vtiles = [vw_sb[:, t, :, :] for t in range(3)]
            # qk[p, n, i]: n=0,1 -> q cin halves, n=2,3 -> k cin halves
            qk_sb = persist.tile([128, 4, S], BF16, name="qk_sb")
            nc.gpsimd.dma_start(out=qk_sb[:, 0:2, :], in_=qk_d[:, 0:2, :])
            nc.sync.dma_start(out=qk_sb[:, 2:4, :], in_=qk_d[:, 2:4, :])
            qT = [qk_sb[:, 0, :], qk_sb[:, 1, :]]
            kTt = [qk_sb[:, 2, :], qk_sb[:, 3, :]]
            gb_sb = const.tile([128, 20], F32, name="gb_sb")
            nc.scalar.dma_start(out=gb_sb, in_=gb_d)
            g_sbs = [gb_sb[:, 0:4], gb_sb[:, 4:8], gb_sb[:, 8:12]]
            bq_sb = gb_sb[:, 12:16]
            bk_sb = gb_sb[:, 16:20]
            bvb = const.tile([128, C], BF16, name="bvb")
            nc.gpsimd.dma_start(
                out=bvb,
                in_=bass.AP(tensor=bv_d.tensor, offset=bv_d.offset,
                            ap=[[0, 128]] + list(bv_d.ap)),
            )

            wT = [None, None, None]   # wT[t][m]: [128, 512] bf16
            scales = []               # per-weight [128, 4] fp32 scale tiles
            QT, KT, VP = [], [], []
            OUT = [persist.tile([128, C], BF16, tag=f"OUT{i}", name=f"OUT{i}")
                   for i in range(8)]
            for ct in range(4):
                QT.append(persist.tile([128, S], BF16, tag=f"QT{ct}", name=f"QT{ct}"))
                KT.append(persist.tile([128, S], BF16, tag=f"KT{ct}", name=f"KT{ct}"))
            for st in range(8):
                VP.append(persist.tile([128, NH * 65], BF16, tag=f"VP{st}",
                                       name=f"VP{st}"))

            scales = [const.tile([128, 4], F32, name=f"scale_sb{t}")
                      for t in range(3)]

            # Weight-norm scales, batched [128, 12] (column 4t+c):
            # squares on Pool (tensor_tensor; ACT must stay exp-only and both
            # DVE tensor_tensor_reduce and Pool tensor_scalar fault real hw),
            # free-dim reduce on DVE, sqrt on ACT, recip/xg on DVE.
            ssum12 = const.tile([128, 12], F32, name="ssum12")
            snorm12 = const.tile([128, 12], F32, name="snorm12")
            scale12 = const.tile([128, 12], F32, name="scale12")
            scales = [scale12[:, 4 * t:4 * t + 4] for t in range(3)]

            # contiguous squares buffer for the 10 staggered chains: slots
            # [v0..v3, q1..q3, k1..k3]; reduced in 3 batched DVE ops whose
            # late readiness keeps the scheduler from hoisting them ahead of
            # the projection epilogues on DVE
            sq10 = persist.tile([128, 10, CIN], BF16, name="sq10")
            _SLOT = {(2, 0): 0, (2, 1): 1, (2, 2): 2, (2, 3): 3,
                     (0, 1): 4, (0, 2): 5, (0, 3): 6,
                     (1, 1): 7, (1, 2): 8, (1, 3): 9}

            def emit_sq_act(t_i, c):
                # ACT Square+accum: used only for the two pre-loop chains
                # (q0/k0), emitted before the wT copies so the scale is ready
                # by the first projection epilogue
                vt = vtiles[t_i][:, c, :]
                sqv = wtmp.tile([128, CIN], BF16, tag="sqv", bufs=2,
                                name=f"sqva{t_i}_{c}")
                col = 4 * t_i + c
                nc.scalar.activation(out=sqv, in_=vt, func=AF.Square,
                                     accum_out=ssum12[:, col:col + 1])
                nc.scalar.activation(out=snorm12[:, col:col + 1],
                                     in_=ssum12[:, col:col + 1], func=AF.Sqrt)
                rr = smalls.tile([128, 1], F32, tag="rs", name=f"rsa{col}")
                nc.vector.reciprocal(rr, snorm12[:, col:col + 1])
                nc.vector.tensor_mul(scale12[:, col:col + 1], rr,
                                     gb_sb[:, col:col + 1])

            def emit_sq10(t_i, c):
                vt = vtiles[t_i][:, c, :]
                nc.gpsimd.tensor_mul(sq10[:, _SLOT[(t_i, c)], :], vt, vt)

            def emit_red10():
                # v -> cols 8:12, q1-3 -> cols 1:4, k1-3 -> cols 5:8
                nc.vector.tensor_reduce(
                    out=ssum12[:, 8:12], in_=sq10[:, 0:4, :],
                    axis=mybir.AxisListType.X, op=ALU.add)
                nc.vector.tensor_reduce(
                    out=ssum12[:, 1:4], in_=sq10[:, 4:7, :],
                    axis=mybir.AxisListType.X, op=ALU.add)
                nc.vector.tensor_reduce(
                    out=ssum12[:, 5:8], in_=sq10[:, 7:10, :],
                    axis=mybir.AxisListType.X, op=ALU.add)

            def emit_scale_fin(c0, cn):
                # sqrt+recip+xg for ssum12 columns [c0, c0+cn)
                nc.scalar.activation(out=snorm12[:, c0:c0 + cn],
                                     in_=ssum12[:, c0:c0 + cn], func=AF.Sqrt)
                rr = smalls.tile([128, cn], F32, tag="rs", name=f"rs{c0}")
                nc.vector.reciprocal(rr, snorm12[:, c0:c0 + cn])
                nc.vector.tensor_mul(scale12[:, c0:c0 + cn], rr,
                                     gb_sb[:, c0:c0 + cn])

            def emit_transposes(pool, t_i, diag=None, act_copies=False):
                # shares the rotating "pp" psum buffers with the projections
                wp = [
                    pool.tile([128, 512], F32, tag="pp", bufs=2,
                              name=f"wp{m}_{t_i}")
                    for m in range(2)
                ]
                for c in range(4):
                    vt = vtiles[t_i][:, c, :]
                    rhs_t = eye_bf if diag is None else diag[c]
                    for m in range(2):
                        nc.tensor.matmul(
                            wp[m][:, 128 * c:128 * (c + 1)],
                            lhsT=vt[:, 128 * m:128 * (m + 1)],
                            rhs=rhs_t,
                            start=True, stop=True,
                        )
                pair = []
                for m in range(2):
                    wTm = persist.tile([128, C], BF16, tag=f"wT{t_i}_{m}",
                                       name=f"wT{t_i}_{m}")
                    if act_copies:
                        # ACT is idle pre-exp; frees DVE for the reductions
                        nc.scalar.activation(out=wTm, in_=wp[m], func=AF.Copy)
                    else:
                        nc.vector.tensor_copy(out=wTm, in_=wp[m])
                    pair.append(wTm)
                wT[t_i] = pair

            def emit_sq(t_i, c):
                vt = vtiles[t_i][:, c, :]
                sqv = wtmp.tile([128, CIN], BF16, tag="sqv", bufs=2,
                                name=f"sqv{t_i}_{c}")
                nc.gpsimd.tensor_mul(sqv, vt, vt)
                col = 4 * t_i + c
                nc.vector.tensor_reduce(
                    out=ssum12[:, col:col + 1], in_=sqv,
                    axis=mybir.AxisListType.X, op=ALU.add)

            # up front: only the c=0 squares (needed by proj(0)'s epilogue)
            emit_sq(0, 0)
            emit_sq(1, 0)

            with tc.tile_pool(name="psW", bufs=1, space="PSUM") as psW:
                emit_transposes(psW, 0, act_copies=True)
                emit_transposes(psW, 1, act_copies=True)
                emit_scale_fin(0, 1)
                emit_scale_fin(4, 1)
                def emit_proj_group(ct, g):
                    # g in 0..3 -> (q/k, n-half)
                    dst, wpair, src, scale_sb, b_sb, pnm = (
                        (QT, wT[0], qT, scales[0], bq_sb, "q"),
                        (KT, wT[1], kTt, scales[1], bk_sb, "k"),
                    )[g // 2]
                    n = g % 2
                    pp = psW.tile([128, 512], F32, tag="pp", bufs=2,
                                  name=f"pp{pnm}{ct}_{n}")
                    for kc in range(2):
                        nc.tensor.matmul(
                            pp,
                            lhsT=wpair[kc][:, 128 * ct:128 * (ct + 1)],
                            rhs=src[kc][:, 512 * n:512 * (n + 1)],
                            start=(kc == 0), stop=(kc == 1),
                        )
                    # fused weight-norm scale + bias epilogue (DVE: Pool
                    # cannot access PSUM, ACT must stay exp-only)
                    nc.vector.tensor_scalar(
                        out=dst[ct][:, 512 * n:512 * (n + 1)],
                        in0=pp,
                        scalar1=scale_sb[:, ct:ct + 1],
                        scalar2=b_sb[:, ct:ct + 1],
                        op0=ALU.mult, op1=ALU.add,
                    )

                def emit_proj(ct):
                    for g in range(4):
                        emit_proj_group(ct, g)

                def emit_v(st):
                    vp = VP[st]
                    ppv = psW.tile([128, 512], F32, tag="pp", bufs=2, name=f"ppv{st}")
                    for kc in range(2):
                        nc.tensor.matmul(
                            ppv,
                            lhsT=kTt[kc][:, 128 * st:128 * (st + 1)],
                            rhs=wT[2][kc],
                            start=(kc == 0), stop=(kc == 1),
                        )
                    vp3 = vp.rearrange("p (h c) -> p h c", c=65)
                    nc.gpsimd.memset(vp3[:, :, 64:65], 1.0)
                    nc.vector.tensor_add(
                        vp3[:, :, 0:64],
                        ppv.rearrange("p (h c) -> p h c", c=64),
                        bvb.rearrange("p (h c) -> p h c", c=64),
                    )

                with (
                    tc.tile_pool(name="psL", bufs=1, space="PSUM") as psL,
                    tc.tile_pool(name="psPV", bufs=2, space="PSUM") as psPV,
                    tc.tile_pool(name="epool", bufs=2) as epool,
                ):
                    def emit_L(a2, j, eTs):
                        # j >= 4: two consecutive j's share one psum tile and
                        # one exp per head (ACT per-op overhead is 352 cycles)
                        js = [j] if j < 4 else [j, j + 1]
                        njs_ = [S - 128 * jj for jj in js]
                        w = sum(njs_)
                        e = epool.tile([128, 2 * w], BF16, tag=f"e_{j}",
                                       name=f"e_{a2}_{j}")
                        offs = []   # per j in js: (off_h0, off_h1)
                        o = 0
                        for nj_ in njs_:
                            offs.append((o, w + o))
                            o += nj_
                        for jj, (o0, _o1) in zip(js, offs):
                            eTs.append((e, offs[js.index(jj)]))
                        for hi in range(2):
                            p0 = 64 * hi
                            lt = psL.tile([128, w], F32, tag=f"lt{hi}",
                                          name=f"lt{hi}_{a2}_{j}")
                            base = 0
                            for jj, nj_ in zip(js, njs_):
                                for c0 in range(0, nj_, 512):
                                    cw = min(512, nj_ - c0)
                                    nc.tensor.matmul(
                                        lt[:, base + c0:base + c0 + cw],
                                        lhsT=KT[a2][p0:p0 + 64,
                                                    128 * jj:128 * jj + 128],
                                        rhs=QT[a2][p0:p0 + 64,
                                                   128 * jj + c0:128 * jj + c0 + cw],
                                        start=True, stop=True,
                                    )
                                base += nj_
                            nc.scalar.activation(
                                out=e[:, hi * w:hi * w + w], in_=lt,
                                func=AF.Exp, scale=0.125)
                            for (o0, o1) in offs:
                                off = o0 if hi == 0 else o1
                                # SBUF->SBUF bf16 on the idle Pool engine
                                nc.gpsimd.tensor_mul(
                                    e[:, off:off + 128],
                                    e[:, off:off + 128], triu)

                    def emit_PV(a2, i, eTs):
                        # both heads accumulate into one 1-bank psum tile
                        po = psPV.tile([128, 130], F32, tag="po",
                                       name=f"po_{a2}_{i}")
                        for hi in range(2):
                            hh = 2 * a2 + hi
                            for jj in range(i + 1):
                                et, (o0, o1) = eTs[jj]
                                base = (o0, o1)[hi] + 128 * (i - jj)
                                nc.tensor.matmul(
                                    po[:, 65 * hi:65 * hi + 65],
                                    lhsT=et[:, base:base + 128],
                                    rhs=VP[jj][:, 65 * hh:65 * hh + 65],
                                    start=(jj == 0), stop=(jj == i),
                                )
                        po3 = po.rearrange("p (g x) -> p g x", g=2)
                        r = smalls.tile([128, 2], F32, tag="r",
                                        name=f"r{a2}_{i}")
                        nc.vector.reciprocal(r, po3[:, :, 64:65])
                        for hi in range(2):
                            hh = 2 * a2 + hi
                            nc.vector.tensor_scalar_mul(
                                out=OUT[i][:, 64 * hh:64 * hh + 64],
                                in0=po[:, 65 * hi:65 * hi + 64],
                                scalar1=r[:, hi:hi + 1],
                            )

                    store_rings = [nc.sync, nc.gpsimd]

                    def store_out(i):
                        store_rings[i % 2].dma_start(
                            out=o_d[128 * i:128 * (i + 1), :], in_=OUT[i])

                    prev_eTs = None
                    diag = []
                    for a2 in range(4):
                        emit_proj(a2)
                        if a2 == 0:
                            # remaining norm squares on Pool (V first: its
                            # scales gate the diag + t2 transposes)
                            for c in range(4):
                                emit_sq10(2, c)
                            for c in range(1, 4):
                                emit_sq10(0, c)
                                emit_sq10(1, c)
                            emit_red10()
                        eTs = []
                        for j in range(8):
                            if j not in (5, 7):
                                emit_L(a2, j, eTs)
                            if a2 == 0:
                                # V diag/transposes/projection staggered
                                # through the first pass as ACT/PE filler
                                if j == 2:
                                    emit_scale_fin(8, 4)
                                elif j == 3:
                                    for c in range(4):
                                        rhs_t = wtmp.tile(
                                            [128, 128], BF16, tag="diag",
                                            bufs=4, name=f"diag2_{c}")
                                        nc.vector.tensor_scalar_mul(
                                            out=rhs_t, in0=eye_bf,
                                            scalar1=scales[2][:, c:c + 1])
                                        diag.append(rhs_t)
                                elif j == 4:
                                    emit_transposes(psW, 2, diag=diag)
                                elif j == 5:
                                    # q/k scales for column-tiles 1-3
                                    emit_scale_fin(1, 3)
                                    emit_scale_fin(5, 3)
                                if 4 <= j <= 7:
                                    emit_v(2 * (j - 4))
                                    emit_v(2 * (j - 4) + 1)
                            if prev_eTs is not None:
                                # descending: biggest PV first covers the
                                # biggest exp; VP[jj<=7-j] all ready by a2=1
                                emit_PV(a2 - 1, 7 - j, prev_eTs)
                        prev_eTs = eTs
                    # a2=3 PV drain after all logits are in flight: ascending
                    # so PV(3,0) (oldest e-tiles) starts while the last exps
                    # and masks finish
                    for i in range(8):
                        emit_PV(3, i, prev_eTs)
                        if i == 0:
                            # query row 0 attends to nothing; its PV rows are
                            # 0*inf=NaN until zeroed (all pairs done by now)
                            nc.vector.memset(OUT[0][0:1, :], 0.0)
                        store_out(i)
    nc.compile()
    return nc


_CACHE = {}


def _get_module():
    if "nc" not in _CACHE:
        _CACHE["nc"] = _build_module()
    return _CACHE["nc"]


def _in_maps(inputs):
    import ml_dtypes

    q = np.asarray(inputs["query"], dtype=np.float32)
    k = np.asarray(inputs["key"], dtype=np.float32)
    B = q.shape[0]
    assert B == N_CORES
    vw = np.concatenate(
        [np.asarray(inputs[f"v{nm}"], np.float32) for nm in ("q", "k", "v")],
        axis=0).astype(ml_dtypes.bfloat16)          # [1536, 256]
    vw = np.ascontiguousarray(
        vw.reshape(3, 4, 128, CIN).transpose(2, 0, 1, 3))   # [128, 3, 4, 256]
    gb = np.stack(
        [np.asarray(inputs["gq"], np.float32),
         np.asarray(inputs["gk"], np.float32),
         np.asarray(inputs["gv"], np.float32),
         np.asarray(inputs["bq"], np.float32),
         np.asarray(inputs["bk"], np.float32)])     # [5, 512]
    gb = np.ascontiguousarray(
        gb.reshape(5, 4, 128).transpose(2, 0, 1).reshape(128, 20))
    bv = np.ascontiguousarray(
        np.asarray(inputs["bv"], np.float32).astype(ml_dtypes.bfloat16))
    eye = np.eye(128, dtype=ml_dtypes.bfloat16)
    triu = np.triu(np.ones((128, 128), np.float32), k=1).astype(ml_dtypes.bfloat16)
    msk = np.ascontiguousarray(np.concatenate([eye, triu], axis=1))
    shared = {"vw": vw, "gb": gb, "bv": bv, "msk": msk}
    maps = []
    for b in range(B):
        m = dict(shared)
        qk = np.concatenate(
            [q[b].reshape(CIN, S), k[b].reshape(CIN, S)], axis=0
        ).astype(ml_dtypes.bfloat16)                # [512, 1024]
        m["qk"] = np.ascontiguousarray(
            qk.reshape(4, 128, S).transpose(1, 0, 2))   # [128, 4, 1024]
        maps.append(m)
    return maps


def _gather(results):
    outs = []
    for b in range(N_CORES):
        o = results[b]["o"]                       # [S, C] bf16
        outs.append(np.ascontiguousarray(o.T).astype(np.float32).reshape(C, HW, HW))
    return np.stack(outs)                          # [B, C, H, W] fp32


def run(inputs, **kw):
    """Run on hardware; returns (full_output, BassKernelResults)."""
    nc = _get_module()
    res = run_bass_kernel_spmd(nc, _in_maps(inputs), list(range(N_CORES)), **kw)
    return _gather(res.results), res


def kernel(**inputs):
    out, _ = run(inputs)
    return out
